# revision 5
# baseline (speedup 1.0000x reference)
"""Trainium2 Bass kernel for nn_CNN_88098369175781.

Model: x[1,1,18,T=262144] -> wavA=x[...,0,:], eeg=x[...,1:17,:], wavB=x[...,17,:]
  wav streams: proj(1->16, pointwise) -> diagonal sinc filter bank (15 taps,
  pad 7) -> conv(16->10, 9 taps) + bias -> relu -> global max-pool.
  eeg stream:  conv(16->10, 9 taps) + bias -> relu -> global max-pool.
  concat -> sigmoid FC(30->30) -> sigmoid FC(30->2).

Device decomposition (validated vs reference to ~2e-7 rel err in numpy):
  * Each wav stream's three linear stages fuse into ONE 1->10 channel, 23-tap
    conv on the zero-padded raw wav signal (weights precomposed on host).
  * Bias/relu commute past the global max (bias is constant over time;
    max(relu(h)) = relu(max(h))), so the device only computes convs + maxima.
  * Convs run on the tensor engine via a polyphase formulation:
      eeg:  time phases r in [0,8), outputs (o, dt in [0,8)) => M=80,
            contraction (c,r) => K=128, 2 accumulating matmuls (u-groups).
      wav:  time phases v in [0,12), outputs (o, dt in [0,12)) => M=120,
            contraction (v,q in [0,3)) => K=36, single matmul per tile
            (the q-replication is materialized host-side).
  * 8 cores split the time axis (overlapping chunks; overlap is free for max).
  * Host combines per-core maxima (320 floats/core) and runs the tiny FC head.
"""

import os
import numpy as np

T = 262144
NOUT = T - 8            # 262136 valid conv output positions
NCORES = 8
KLEN = 15
SIGMA = 0.005

EEG_NCOL = 4096         # eeg matmul columns per core (8 outputs each)
EEG_COLS = EEG_NCOL + 1  # phase row length (g=1 needs one extra column)
WAV_NCOL = 2731         # wav matmul columns per core (12 outputs each)
EEG_TC = 8 * EEG_NCOL   # 32768 eeg outputs per core
WAV_TC = 12 * WAV_NCOL  # 32772 wav outputs per core

_NC_CACHE = {}
LAST_RESULT = None      # BassKernelResults of the most recent device run


# --------------------------------------------------------------------------
# host-side weight precompute
# --------------------------------------------------------------------------

def _sinc_rows(mu):
    """Diagonal rows of the reference's sinc_kernel: [16, 15] float64."""
    k = np.linspace(-1.0, 1.0, KLEN)
    kk = (k[None, :] - np.asarray(mu, np.float64)[:, None]) / SIGMA
    nos = np.sum(np.abs(kk) < 1e-5, axis=1)
    kk = np.where((nos >= 0.5)[:, None], kk - 5e-5, kk)
    return np.sin(np.pi * kk) / (np.pi * kk)


def _composite_wav_weights(mu, proj_w, conv_w_i):
    """Fused 1->10ch 23-tap kernel E[o, s] (float64)."""
    krn = _sinc_rows(mu)                                  # [16,15]
    a = np.asarray(proj_w, np.float64)[:, 0, 0]           # [16]
    W = np.asarray(conv_w_i, np.float64)                  # [10,16,9]
    E = np.zeros((10, 23))
    for j in range(9):
        E[:, j:j + 15] += np.einsum('oc,cm->om', W[:, :, j] * a[None, :], krn)
    return E


def _eeg_lhsT(W1):
    """[128, 160]: cols g*80+(o*8+dt); row c*8+r; val W1[o,c,8g+r-dt]."""
    W1 = np.asarray(W1, np.float64)
    out = np.zeros((128, 160))
    g, c, r, o, dt = np.meshgrid(np.arange(2), np.arange(16), np.arange(8),
                                 np.arange(10), np.arange(8), indexing='ij')
    j = 8 * g + r - dt
    valid = (j >= 0) & (j < 9)
    out[(c * 8 + r)[valid], (g * 80 + o * 8 + dt)[valid]] = \
        W1[o[valid], c[valid], np.clip(j[valid], 0, 8)]
    return out.astype(np.float32)


def _wav_lhsT(E):
    """[36, 120]: row v*3+q, col o*12+dt, val E[o, 12q+v-dt]."""
    out = np.zeros((36, 120))
    v, q, o, dt = np.meshgrid(np.arange(12), np.arange(3), np.arange(10),
                              np.arange(12), indexing='ij')
    s = 12 * q + v - dt
    valid = (s >= 0) & (s < 23)
    out[(v * 3 + q)[valid], (o * 12 + dt)[valid]] = E[o[valid], np.clip(s[valid], 0, 22)]
    return out.astype(np.float32)


# --------------------------------------------------------------------------
# host-side per-core input slicing
# --------------------------------------------------------------------------

def _core_starts(k):
    return (min(k * 32767, NOUT - EEG_TC), min(k * 32767, NOUT - WAV_TC))


def _eeg_phases(eeg, k):
    """[128, 4097]: row c*8+r, col m = eeg[c, s_e + 8m + r]."""
    s_e, _ = _core_starts(k)
    v = eeg[:, s_e:s_e + 8 * EEG_COLS]                  # [16, 32776]
    p = v.reshape(16, EEG_COLS, 8).transpose(0, 2, 1)   # [16,8,4097]
    return np.ascontiguousarray(p.reshape(128, EEG_COLS), dtype=np.float32)


def _wav_phases(w_pad, k):
    """[36, 2731]: row v*3+q, col n = w_pad[s_w + 12(n+q) + v]."""
    _, s_w = _core_starts(k)
    sl = w_pad[s_w:s_w + 12 * (WAV_NCOL + 2)]
    y = sl.reshape(WAV_NCOL + 2, 12).T                  # y[v,m] = sl[12m+v]
    out = np.empty((36, WAV_NCOL), dtype=np.float32)
    for q in range(3):
        out[q::3, :] = y[:, q:q + WAV_NCOL]
    return out


# --------------------------------------------------------------------------
# bass kernel
# --------------------------------------------------------------------------

def _build_nc():
    import concourse.bacc as bacc
    import concourse.bass as bass
    import concourse.tile as tile
    import concourse.mybir as mybir

    f32 = mybir.dt.float32
    nc = bacc.Bacc("TRN2", target_bir_lowering=False, debug=False,
                   num_devices=NCORES)

    eegP = nc.dram_tensor("eegP", [128, EEG_COLS], f32, kind="ExternalInput")
    wavPA = nc.dram_tensor("wavPA", [36, WAV_NCOL], f32, kind="ExternalInput")
    wavPB = nc.dram_tensor("wavPB", [36, WAV_NCOL], f32, kind="ExternalInput")
    wE = nc.dram_tensor("wE", [128, 160], f32, kind="ExternalInput")
    wW = nc.dram_tensor("wW", [36, 240], f32, kind="ExternalInput")
    out = nc.dram_tensor("out", [320], f32, kind="ExternalOutput")

    N_ECHUNK = 4                 # eeg input loaded in 4 column chunks
    ECHUNK = 1024                # chunk j covers cols [1024j, 1024j+1025)
    EEG_NT = 8                   # 8 eeg matmul tiles of N=512
    WAV_NT = 6                   # per wav stream: 5x512 + 171

    with tile.TileContext(nc) as tc:
        with (
            tc.tile_pool(name="const", bufs=1) as cpool,
            tc.tile_pool(name="eegc", bufs=N_ECHUNK) as epool,
            tc.tile_pool(name="wavs", bufs=1) as wpool,
            tc.tile_pool(name="mx", bufs=1) as mpool,
            tc.tile_pool(name="pse", bufs=2, space="PSUM") as psE,
            tc.tile_pool(name="psw", bufs=2, space="PSUM") as psW,
        ):
            wE_t = cpool.tile([128, 160], f32, tag="wE")
            wW_t = cpool.tile([36, 240], f32, tag="wW")
            nc.sync.dma_start(wE_t[:], wE[:])
            nc.sync.dma_start(wW_t[:], wW[:])

            wavA_t = wpool.tile([36, WAV_NCOL], f32, tag="wavA")
            wavB_t = wpool.tile([36, WAV_NCOL], f32, tag="wavB")

            echunks = []
            for j in range(N_ECHUNK):
                et = epool.tile([128, ECHUNK + 1], f32, tag="eegchunk")
                lo = j * ECHUNK
                nc.sync.dma_start(et[:], eegP[:, lo:lo + ECHUNK + 1])
                echunks.append(et)
                if j == 1:
                    nc.sync.dma_start(wavA_t[:], wavPA[:])
                if j == 2:
                    nc.sync.dma_start(wavB_t[:], wavPB[:])

            mE = mpool.tile([80, 4], f32, tag="mE")
            mA = mpool.tile([120, 4], f32, tag="mA")
            mB = mpool.tile([120, 4], f32, tag="mB")

            # eeg: 4 psum pairs, each pair = 2 column tiles x 2 accumulating
            # matmuls, then one 2-bank-wide reduce
            for p in range(4):
                ch = echunks[p]
                ps = psE.tile([80, 2, 512], f32, tag="pse")
                for j in range(2):
                    lo = j * 512
                    nc.tensor.matmul(ps[:, j, :], wE_t[:, 0:80],
                                     ch[:, lo:lo + 512], start=True, stop=False)
                    nc.tensor.matmul(ps[:, j, :], wE_t[:, 80:160],
                                     ch[:, lo + 1:lo + 513], start=False, stop=True)
                nc.vector.reduce_max(mE[:, p:p + 1], ps[:],
                                     axis=mybir.AxisListType.XY)

            # wav streams: 3 psum pairs per stream (tiles of N=512, last 171)
            for wt, mcol, mt in [(wavA_t, slice(0, 120), mA),
                                 (wavB_t, slice(120, 240), mB)]:
                for p in range(3):
                    ps = psW.tile([120, 2, 512], f32, tag="psw")
                    for j in range(2):
                        n0 = (2 * p + j) * 512
                        nn = min(512, WAV_NCOL - n0)
                        nc.tensor.matmul(ps[:, j, :nn], wW_t[:, mcol],
                                         wt[:, n0:n0 + nn], start=True, stop=True)
                    if p < 2:
                        nc.vector.reduce_max(mt[:, p:p + 1], ps[:],
                                             axis=mybir.AxisListType.XY)
                    else:
                        nc.vector.reduce_max(mt[:, 2:3], ps[:, 0, :],
                                             axis=mybir.AxisListType.X)
                        nc.vector.reduce_max(mt[:, 3:4], ps[:, 1, :171],
                                             axis=mybir.AxisListType.X)

            mEf = mpool.tile([80, 1], f32, tag="mEf")
            mAf = mpool.tile([120, 1], f32, tag="mAf")
            mBf = mpool.tile([120, 1], f32, tag="mBf")
            nc.vector.reduce_max(mEf[:], mE[:], axis=mybir.AxisListType.X)
            nc.vector.reduce_max(mAf[:], mA[:], axis=mybir.AxisListType.X)
            nc.vector.reduce_max(mBf[:], mB[:], axis=mybir.AxisListType.X)

            nc.sync.dma_start(out[0:80], mEf[:, 0])
            nc.sync.dma_start(out[80:200], mAf[:, 0])
            nc.sync.dma_start(out[200:320], mBf[:, 0])

    nc.compile()
    return nc


def _get_nc():
    if "nc" not in _NC_CACHE:
        _NC_CACHE["nc"] = _build_nc()
    return _NC_CACHE["nc"]


# --------------------------------------------------------------------------
# entry point
# --------------------------------------------------------------------------

def _prepare_in_maps(x, mu, projA_w, projB_w, conv_w):
    x = np.asarray(x, np.float32)
    eeg = np.ascontiguousarray(x[0, 0, 1:17, :])
    # generous zero tail so every core's reshape window exists
    zt = np.zeros(64, np.float32)
    w_padA = np.concatenate([np.zeros(7, np.float32), x[0, 0, 0, :], zt])
    w_padB = np.concatenate([np.zeros(7, np.float32), x[0, 0, 17, :], zt])

    conv_w = np.asarray(conv_w)
    E_A = _composite_wav_weights(mu, projA_w, conv_w[0])
    E_B = _composite_wav_weights(mu, projB_w, conv_w[2])
    wE_np = _eeg_lhsT(conv_w[1])
    wW_np = np.concatenate([_wav_lhsT(E_A), _wav_lhsT(E_B)], axis=1)

    in_maps = []
    for k in range(NCORES):
        in_maps.append({
            "eegP": _eeg_phases(eeg, k),
            "wavPA": _wav_phases(w_padA, k),
            "wavPB": _wav_phases(w_padB, k),
            "wE": wE_np,
            "wW": wW_np,
        })
    return in_maps


def _head(percore, conv_b, fc1_w, fc1_b, fc2_w, fc2_b):
    m = percore.max(axis=0).astype(np.float64)
    eeg_o = m[0:80].reshape(10, 8).max(axis=1)
    wavA_o = m[80:200].reshape(10, 12).max(axis=1)
    wavB_o = m[200:320].reshape(10, 12).max(axis=1)
    conv_b = np.asarray(conv_b, np.float64)
    f = np.concatenate([np.maximum(wavA_o + conv_b[0], 0.0),
                        np.maximum(eeg_o + conv_b[1], 0.0),
                        np.maximum(wavB_o + conv_b[2], 0.0)])
    h = 1.0 / (1.0 + np.exp(-(f @ np.asarray(fc1_w, np.float64).T
                              + np.asarray(fc1_b, np.float64))))
    o = 1.0 / (1.0 + np.exp(-(h @ np.asarray(fc2_w, np.float64).T
                              + np.asarray(fc2_b, np.float64))))
    return o[None, :].astype(np.float32)


def kernel(x, mu, projA_w, projB_w, conv_w, conv_b, fc1_w, fc1_b, fc2_w, fc2_b):
    global LAST_RESULT
    in_maps = _prepare_in_maps(x, mu, projA_w, projB_w, conv_w)
    nc = _get_nc()

    if os.environ.get("KERNEL_USE_SIM"):
        # sim mode for correctness checking without hardware
        from concourse.bass_interp import CoreSim
        percore = np.zeros((NCORES, 320), np.float32)
        for k in range(NCORES):
            sim = CoreSim(nc)
            for name, arr in in_maps[k].items():
                sim.tensor(name)[:] = arr
            sim.simulate()
            percore[k] = np.asarray(sim.tensor("out"))
    else:
        from concourse.bass_utils import run_bass_kernel_spmd
        trace = bool(os.environ.get("KERNEL_TRACE"))
        res = run_bass_kernel_spmd(nc, in_maps, list(range(NCORES)),
                                   trace=trace)
        LAST_RESULT = res
        percore = np.stack([res.results[k]["out"] for k in range(NCORES)])

    return _head(percore, conv_b, fc1_w, fc1_b, fc2_w, fc2_b)


# revision 8
# speedup vs baseline: 1.6617x; 1.6617x over previous
"""Trainium2 Bass kernel for nn_CNN_88098369175781.

Model: x[1,1,18,T=262144] -> wavA=x[...,0,:], eeg=x[...,1:17,:], wavB=x[...,17,:]
  wav streams: proj(1->16, pointwise) -> diagonal sinc filter bank (15 taps,
  pad 7) -> conv(16->10, 9 taps) + bias -> relu -> global max-pool.
  eeg stream:  conv(16->10, 9 taps) + bias -> relu -> global max-pool.
  concat -> sigmoid FC(30->30) -> sigmoid FC(30->2).

Device decomposition (validated vs reference in numpy):
  * Each wav stream's three linear stages fuse into ONE 1->10 channel, 23-tap
    conv on the zero-padded raw wav signal (weights precomposed on host).
  * Bias/relu commute past the global max (bias is constant over time;
    max(relu(h)) = relu(max(h))), so the device only computes convs + maxima.
  * Convs run on the tensor engine via a polyphase formulation:
      eeg:  time phases r in [0,8), outputs (o, dt in [0,8)) => M=80,
            contraction (c,r) => K=128, 2 accumulating matmuls (u-groups).
      wav:  time phases v in [0,12), outputs (o, dt in [0,12)) => M=120,
            contraction (v,q in [0,3)) => K=36, single matmul per tile
            (the q-replication is materialized host-side).
  * Operands are fp16 (PSUM accumulation stays fp32): fp32 matmuls run as two
    HW passes on trn2, fp16 single-pass -- and DMA bytes halve. Validated
    rel err ~4e-6 vs the fp32 reference.
  * Max-reduction is split across engines: the vector engine reduces eeg PSUM
    directly (fp32); the scalar engine casts wav PSUM to fp16 in SBUF and the
    vector engine reduces those at the 2-byte 2x mode.
  * 8 cores split the time axis (overlapping chunks; overlap is free for max).
  * Host combines per-core maxima and runs the tiny FC head.
"""

import os
import numpy as np

T = 262144
NOUT = T - 8            # 262136 valid conv output positions
NCORES = 8
KLEN = 15
SIGMA = 0.005

EEG_NCOL = 4096         # eeg matmul columns per core (8 outputs each)
EEG_COLS = EEG_NCOL + 1  # phase row length (g=1 needs one extra column)
WAV_NCOL = 2731         # wav matmul columns per core (12 outputs each)
EEG_TC = 8 * EEG_NCOL   # 32768 eeg outputs per core
WAV_TC = 12 * WAV_NCOL  # 32772 wav outputs per core

_NC_CACHE = {}
LAST_RESULT = None      # BassKernelResults of the most recent device run


# --------------------------------------------------------------------------
# host-side weight precompute
# --------------------------------------------------------------------------

def _sinc_rows(mu):
    """Diagonal rows of the reference's sinc_kernel: [16, 15] float64."""
    k = np.linspace(-1.0, 1.0, KLEN)
    kk = (k[None, :] - np.asarray(mu, np.float64)[:, None]) / SIGMA
    nos = np.sum(np.abs(kk) < 1e-5, axis=1)
    kk = np.where((nos >= 0.5)[:, None], kk - 5e-5, kk)
    return np.sin(np.pi * kk) / (np.pi * kk)


def _composite_wav_weights(mu, proj_w, conv_w_i):
    """Fused 1->10ch 23-tap kernel E[o, s] (float64)."""
    krn = _sinc_rows(mu)                                  # [16,15]
    a = np.asarray(proj_w, np.float64)[:, 0, 0]           # [16]
    W = np.asarray(conv_w_i, np.float64)                  # [10,16,9]
    E = np.zeros((10, 23))
    for j in range(9):
        E[:, j:j + 15] += np.einsum('oc,cm->om', W[:, :, j] * a[None, :], krn)
    return E


def _eeg_lhsT(W1):
    """[128, 160]: cols g*80+(o*8+dt); row c*8+r; val W1[o,c,8g+r-dt]."""
    W1 = np.asarray(W1, np.float64)
    out = np.zeros((128, 160))
    g, c, r, o, dt = np.meshgrid(np.arange(2), np.arange(16), np.arange(8),
                                 np.arange(10), np.arange(8), indexing='ij')
    j = 8 * g + r - dt
    valid = (j >= 0) & (j < 9)
    out[(c * 8 + r)[valid], (g * 80 + o * 8 + dt)[valid]] = \
        W1[o[valid], c[valid], np.clip(j[valid], 0, 8)]
    return out.astype(np.float32)


def _wav_lhsT(E):
    """[36, 120]: row v*3+q, col o*12+dt, val E[o, 12q+v-dt]."""
    out = np.zeros((36, 120))
    v, q, o, dt = np.meshgrid(np.arange(12), np.arange(3), np.arange(10),
                              np.arange(12), indexing='ij')
    s = 12 * q + v - dt
    valid = (s >= 0) & (s < 23)
    out[(v * 3 + q)[valid], (o * 12 + dt)[valid]] = E[o[valid], np.clip(s[valid], 0, 22)]
    return out.astype(np.float32)


# --------------------------------------------------------------------------
# host-side per-core input slicing
# --------------------------------------------------------------------------

def _core_starts(k):
    return (min(k * 32767, NOUT - EEG_TC), min(k * 32767, NOUT - WAV_TC))


def _eeg_phases(eeg, k):
    """[128, 4097]: row c*8+r, col m = eeg[c, s_e + 8m + r]."""
    s_e, _ = _core_starts(k)
    v = eeg[:, s_e:s_e + 8 * EEG_COLS]                  # [16, 32776]
    p = v.reshape(16, EEG_COLS, 8).transpose(0, 2, 1)   # [16,8,4097]
    return p.reshape(128, EEG_COLS)


def _wav_phases(w_pad, k):
    """[36, 2731]: row v*3+q, col n = w_pad[s_w + 12(n+q) + v]."""
    _, s_w = _core_starts(k)
    sl = w_pad[s_w:s_w + 12 * (WAV_NCOL + 2)]
    y = sl.reshape(WAV_NCOL + 2, 12).T                  # y[v,m] = sl[12m+v]
    out = np.empty((36, WAV_NCOL), dtype=w_pad.dtype)
    for q in range(3):
        out[q::3, :] = y[:, q:q + WAV_NCOL]
    return out


# --------------------------------------------------------------------------
# bass kernel
# --------------------------------------------------------------------------

def _build_nc():
    import concourse.bacc as bacc
    import concourse.tile as tile
    import concourse.mybir as mybir

    f32 = mybir.dt.float32
    f16 = mybir.dt.float16
    nc = bacc.Bacc("TRN2", target_bir_lowering=False, debug=False,
                   num_devices=NCORES)

    eegP = nc.dram_tensor("eegP", [128, EEG_COLS], f16, kind="ExternalInput")
    wavPA = nc.dram_tensor("wavPA", [36, WAV_NCOL], f16, kind="ExternalInput")
    wavPB = nc.dram_tensor("wavPB", [36, WAV_NCOL], f16, kind="ExternalInput")
    wE = nc.dram_tensor("wE", [128, 160], f16, kind="ExternalInput")
    wW = nc.dram_tensor("wW", [36, 240], f16, kind="ExternalInput")
    out = nc.dram_tensor("out", [128, 3], f32, kind="ExternalOutput")

    N_ECHUNK = 4                 # eeg input loaded in 4 column chunks
    ECHUNK = 1024                # chunk j covers cols [1024j, 1024j+1025)

    with tile.TileContext(nc) as tc:
        with (
            tc.tile_pool(name="const", bufs=1) as cpool,
            tc.tile_pool(name="eegc", bufs=N_ECHUNK) as epool,
            tc.tile_pool(name="wavs", bufs=1) as wpool,
            tc.tile_pool(name="stg", bufs=2) as spool,
            tc.tile_pool(name="mx", bufs=1) as mpool,
            tc.tile_pool(name="pse", bufs=2, space="PSUM") as psE,
            tc.tile_pool(name="psw", bufs=2, space="PSUM") as psW,
        ):
            wE_t = cpool.tile([128, 160], f16, tag="wE")
            wW_t = cpool.tile([36, 240], f16, tag="wW")
            nc.sync.dma_start(wE_t[:], wE[:])
            nc.sync.dma_start(wW_t[:], wW[:])

            wavA_t = wpool.tile([36, WAV_NCOL], f16, tag="wavA")
            wavB_t = wpool.tile([36, WAV_NCOL], f16, tag="wavB")

            echunks = []
            for j in range(N_ECHUNK):
                et = epool.tile([128, ECHUNK + 1], f16, tag="eegchunk")
                lo = j * ECHUNK
                nc.sync.dma_start(et[:], eegP[:, lo:lo + ECHUNK + 1])
                echunks.append(et)
                if j == 1:
                    nc.sync.dma_start(wavA_t[:], wavPA[:])
                if j == 2:
                    nc.sync.dma_start(wavB_t[:], wavPB[:])

            mE = mpool.tile([80, 4], f32, tag="mE")
            mA = mpool.tile([120, 3], f16, tag="mA")
            mB = mpool.tile([120, 3], f16, tag="mB")

            # eeg: 4 psum pairs; vector engine reduces fp32 straight from PSUM
            for p in range(4):
                ch = echunks[p]
                ps = psE.tile([80, 1024], f32, tag="pse")
                for j in range(2):
                    lo = j * 512
                    nc.tensor.matmul(ps[:, lo:lo + 512], wE_t[:, 0:80],
                                     ch[:, lo:lo + 512], start=True, stop=False)
                    nc.tensor.matmul(ps[:, lo:lo + 512], wE_t[:, 80:160],
                                     ch[:, lo + 1:lo + 513], start=False, stop=True)
                nc.vector.reduce_max(mE[:, p:p + 1], ps[:],
                                     axis=mybir.AxisListType.X)

            # wav streams: scalar engine casts PSUM->fp16 SBUF, vector engine
            # reduces the fp16 tiles (2-byte 2x mode)
            Copy = mybir.ActivationFunctionType.Copy
            for wt, mcol, mt in [(wavA_t, slice(0, 120), mA),
                                 (wavB_t, slice(120, 240), mB)]:
                for p in range(3):
                    ps = psW.tile([120, 1024], f32, tag="psw")
                    nvalid = 1024 if p < 2 else 683
                    for j in range(2):
                        n0 = (2 * p + j) * 512
                        nn = min(512, WAV_NCOL - n0)
                        nc.tensor.matmul(ps[:, j * 512:j * 512 + nn], wW_t[:, mcol],
                                         wt[:, n0:n0 + nn], start=True, stop=True)
                    st = spool.tile([120, 1024], f16, tag="stg")
                    nc.scalar.activation(st[:, :nvalid], ps[:, :nvalid], Copy)
                    nc.vector.reduce_max(mt[:, p:p + 1], st[:, :nvalid],
                                         axis=mybir.AxisListType.X)

            fin = mpool.tile([128, 3], f32, tag="fin")
            finh = mpool.tile([128, 2], f16, tag="finh")
            nc.gpsimd.memset(fin[:], 0.0)
            nc.vector.reduce_max(fin[0:80, 0:1], mE[:], axis=mybir.AxisListType.X)
            nc.vector.reduce_max(finh[0:120, 0:1], mA[:], axis=mybir.AxisListType.X)
            nc.vector.reduce_max(finh[0:120, 1:2], mB[:], axis=mybir.AxisListType.X)
            nc.vector.tensor_copy(fin[0:120, 1:3], finh[0:120, :])

            nc.sync.dma_start(out[:], fin[:])

    nc.compile()
    return nc


def _get_nc():
    if "nc" not in _NC_CACHE:
        _NC_CACHE["nc"] = _build_nc()
    return _NC_CACHE["nc"]


# --------------------------------------------------------------------------
# entry point
# --------------------------------------------------------------------------

def _prepare_in_maps(x, mu, projA_w, projB_w, conv_w):
    x = np.asarray(x, np.float32)
    eeg = np.ascontiguousarray(x[0, 0, 1:17, :]).astype(np.float16)
    zt = np.zeros(64, np.float32)
    w_padA = np.concatenate([np.zeros(7, np.float32), x[0, 0, 0, :], zt]
                            ).astype(np.float16)
    w_padB = np.concatenate([np.zeros(7, np.float32), x[0, 0, 17, :], zt]
                            ).astype(np.float16)

    conv_w = np.asarray(conv_w)
    E_A = _composite_wav_weights(mu, projA_w, conv_w[0])
    E_B = _composite_wav_weights(mu, projB_w, conv_w[2])
    wE_np = _eeg_lhsT(conv_w[1]).astype(np.float16)
    wW_np = np.concatenate([_wav_lhsT(E_A), _wav_lhsT(E_B)],
                           axis=1).astype(np.float16)

    in_maps = []
    for k in range(NCORES):
        in_maps.append({
            "eegP": np.ascontiguousarray(_eeg_phases(eeg, k)),
            "wavPA": np.ascontiguousarray(_wav_phases(w_padA, k)),
            "wavPB": np.ascontiguousarray(_wav_phases(w_padB, k)),
            "wE": wE_np,
            "wW": wW_np,
        })
    return in_maps


def _head(percore, conv_b, fc1_w, fc1_b, fc2_w, fc2_b):
    m = percore.max(axis=0).astype(np.float64)
    eeg_o = m[0:80].reshape(10, 8).max(axis=1)
    wavA_o = m[80:200].reshape(10, 12).max(axis=1)
    wavB_o = m[200:320].reshape(10, 12).max(axis=1)
    conv_b = np.asarray(conv_b, np.float64)
    f = np.concatenate([np.maximum(wavA_o + conv_b[0], 0.0),
                        np.maximum(eeg_o + conv_b[1], 0.0),
                        np.maximum(wavB_o + conv_b[2], 0.0)])
    h = 1.0 / (1.0 + np.exp(-(f @ np.asarray(fc1_w, np.float64).T
                              + np.asarray(fc1_b, np.float64))))
    o = 1.0 / (1.0 + np.exp(-(h @ np.asarray(fc2_w, np.float64).T
                              + np.asarray(fc2_b, np.float64))))
    return o[None, :].astype(np.float32)


def _percore_from_out(arr):
    """Device 'out' [128,3] fp32 -> flat [320] (eeg 80, wavA 120, wavB 120)."""
    arr = np.asarray(arr)
    return np.concatenate([arr[0:80, 0], arr[0:120, 1], arr[0:120, 2]])


def kernel(x, mu, projA_w, projB_w, conv_w, conv_b, fc1_w, fc1_b, fc2_w, fc2_b):
    global LAST_RESULT
    in_maps = _prepare_in_maps(x, mu, projA_w, projB_w, conv_w)
    nc = _get_nc()

    if os.environ.get("KERNEL_USE_SIM"):
        # sim mode for correctness checking without hardware
        from concourse.bass_interp import CoreSim
        percore = np.zeros((NCORES, 320), np.float32)
        for k in range(NCORES):
            sim = CoreSim(nc)
            for name, arr in in_maps[k].items():
                sim.tensor(name)[:] = arr
            sim.simulate()
            percore[k] = _percore_from_out(sim.tensor("out"))
    else:
        from concourse.bass_utils import run_bass_kernel_spmd
        trace = bool(os.environ.get("KERNEL_TRACE"))
        res = run_bass_kernel_spmd(nc, in_maps, list(range(NCORES)),
                                   trace=trace)
        LAST_RESULT = res
        percore = np.stack([_percore_from_out(res.results[k]["out"])
                            for k in range(NCORES)])

    return _head(percore, conv_b, fc1_w, fc1_b, fc2_w, fc2_b)


# revision 13
# speedup vs baseline: 1.6762x; 1.0088x over previous
"""Trainium2 Bass kernel for nn_CNN_88098369175781.

Model: x[1,1,18,T=262144] -> wavA=x[...,0,:], eeg=x[...,1:17,:], wavB=x[...,17,:]
  wav streams: proj(1->16, pointwise) -> diagonal sinc filter bank (15 taps,
  pad 7) -> conv(16->10, 9 taps) + bias -> relu -> global max-pool.
  eeg stream:  conv(16->10, 9 taps) + bias -> relu -> global max-pool.
  concat -> sigmoid FC(30->30) -> sigmoid FC(30->2).

Device decomposition (validated vs reference in numpy):
  * Each wav stream's three linear stages fuse into ONE 1->10 channel, 23-tap
    conv on the zero-padded raw wav signal (weights precomposed on host).
  * Bias/relu commute past the global max (bias is constant over time;
    max(relu(h)) = relu(max(h))), so the device only computes convs + maxima.
  * Convs run on the tensor engine via a polyphase formulation:
      eeg:  time phases r in [0,8), outputs (o, dt in [0,8)) => M=80,
            contraction (c,r) => K=128, 2 accumulating matmuls (u-groups).
      wav:  time phases v in [0,12), outputs (o, dt in [0,12)) => M=120,
            contraction (v,q in [0,3)) => K=36, single matmul per tile
            (the q-replication is materialized host-side).
  * Operands are fp16 (PSUM accumulation stays fp32): fp32 matmuls run as two
    HW passes on trn2, fp16 single-pass -- and DMA bytes halve. Validated
    rel err ~4e-6 vs the fp32 reference.
  * Max-reduction is split across engines: the vector engine reduces eeg PSUM
    directly (fp32); the scalar engine casts wav PSUM to fp16 in SBUF and the
    vector engine reduces those at the 2-byte 2x mode.
  * 8 cores split the time axis (overlapping chunks; overlap is free for max).
  * Host combines per-core maxima and runs the tiny FC head.
"""

import os
import numpy as np

T = 262144
NOUT = T - 8            # 262136 valid conv output positions
NCORES = 8
KLEN = 15
SIGMA = 0.005

EEG_NCOL = 4096         # eeg matmul columns per core (8 outputs each)
EEG_COLS = EEG_NCOL + 1  # phase row length (g=1 needs one extra column)
WAV_NCOL = 2731         # wav matmul columns per core (12 outputs each)
EEG_TC = 8 * EEG_NCOL   # 32768 eeg outputs per core
WAV_TC = 12 * WAV_NCOL  # 32772 wav outputs per core

_NC_CACHE = {}
LAST_RESULT = None      # BassKernelResults of the most recent device run


# --------------------------------------------------------------------------
# host-side weight precompute
# --------------------------------------------------------------------------

def _sinc_rows(mu):
    """Diagonal rows of the reference's sinc_kernel: [16, 15] float64."""
    k = np.linspace(-1.0, 1.0, KLEN)
    kk = (k[None, :] - np.asarray(mu, np.float64)[:, None]) / SIGMA
    nos = np.sum(np.abs(kk) < 1e-5, axis=1)
    kk = np.where((nos >= 0.5)[:, None], kk - 5e-5, kk)
    return np.sin(np.pi * kk) / (np.pi * kk)


def _composite_wav_weights(mu, proj_w, conv_w_i):
    """Fused 1->10ch 23-tap kernel E[o, s] (float64)."""
    krn = _sinc_rows(mu)                                  # [16,15]
    a = np.asarray(proj_w, np.float64)[:, 0, 0]           # [16]
    W = np.asarray(conv_w_i, np.float64)                  # [10,16,9]
    E = np.zeros((10, 23))
    for j in range(9):
        E[:, j:j + 15] += np.einsum('oc,cm->om', W[:, :, j] * a[None, :], krn)
    return E


def _eeg_lhsT(W1):
    """[128, 160]: cols g*80+(o*8+dt); row c*8+r; val W1[o,c,8g+r-dt]."""
    W1 = np.asarray(W1, np.float64)
    out = np.zeros((128, 160))
    g, c, r, o, dt = np.meshgrid(np.arange(2), np.arange(16), np.arange(8),
                                 np.arange(10), np.arange(8), indexing='ij')
    j = 8 * g + r - dt
    valid = (j >= 0) & (j < 9)
    out[(c * 8 + r)[valid], (g * 80 + o * 8 + dt)[valid]] = \
        W1[o[valid], c[valid], np.clip(j[valid], 0, 8)]
    return out.astype(np.float32)


def _wav_lhsT(E):
    """[36, 120]: row v*3+q, col o*12+dt, val E[o, 12q+v-dt]."""
    out = np.zeros((36, 120))
    v, q, o, dt = np.meshgrid(np.arange(12), np.arange(3), np.arange(10),
                              np.arange(12), indexing='ij')
    s = 12 * q + v - dt
    valid = (s >= 0) & (s < 23)
    out[(v * 3 + q)[valid], (o * 12 + dt)[valid]] = E[o[valid], np.clip(s[valid], 0, 22)]
    return out.astype(np.float32)


# --------------------------------------------------------------------------
# host-side per-core input slicing
# --------------------------------------------------------------------------

def _core_starts(k):
    return (min(k * 32767, NOUT - EEG_TC), min(k * 32767, NOUT - WAV_TC))


def _eeg_phases(eeg, k):
    """[128, 4097]: row c*8+r, col m = eeg[c, s_e + 8m + r]."""
    s_e, _ = _core_starts(k)
    v = eeg[:, s_e:s_e + 8 * EEG_COLS]                  # [16, 32776]
    p = v.reshape(16, EEG_COLS, 8).transpose(0, 2, 1)   # [16,8,4097]
    return p.reshape(128, EEG_COLS)


def _wav_phases(w_pad, k):
    """[36, 2731]: row v*3+q, col n = w_pad[s_w + 12(n+q) + v]."""
    _, s_w = _core_starts(k)
    sl = w_pad[s_w:s_w + 12 * (WAV_NCOL + 2)]
    y = sl.reshape(WAV_NCOL + 2, 12).T                  # y[v,m] = sl[12m+v]
    out = np.empty((36, WAV_NCOL), dtype=w_pad.dtype)
    for q in range(3):
        out[q::3, :] = y[:, q:q + WAV_NCOL]
    return out


# --------------------------------------------------------------------------
# bass kernel
# --------------------------------------------------------------------------

def _build_nc():
    import concourse.bacc as bacc
    import concourse.tile as tile
    import concourse.mybir as mybir

    f32 = mybir.dt.float32
    f16 = mybir.dt.float16
    nc = bacc.Bacc("TRN2", target_bir_lowering=False, debug=False,
                   num_devices=NCORES)

    eegP = nc.dram_tensor("eegP", [128, EEG_COLS], f16, kind="ExternalInput")
    wavP = nc.dram_tensor("wavP", [36, 2 * WAV_NCOL], f16, kind="ExternalInput")
    wts = nc.dram_tensor("wts", [128, 400], f16, kind="ExternalInput")
    out = nc.dram_tensor("out", [128, 3], f32, kind="ExternalOutput")

    N_ECHUNK = 4                 # eeg input loaded in 4 column chunks
    ECHUNK = 1024                # chunk j covers cols [1024j, 1024j+1025)
    N_WARM = 10                  # dummy matmuls to warm the PE clock gate

    with tile.TileContext(nc) as tc:
        with (
            tc.tile_pool(name="const", bufs=1) as cpool,
            tc.tile_pool(name="eegc", bufs=N_ECHUNK) as epool,
            tc.tile_pool(name="wavs", bufs=1) as wpool,
            tc.tile_pool(name="stg", bufs=2) as spool,
            tc.tile_pool(name="mx", bufs=1) as mpool,
            tc.tile_pool(name="pse", bufs=2, space="PSUM") as psE,
            tc.tile_pool(name="psw", bufs=2, space="PSUM") as psW,
        ):
            # PE warmup: dummy matmuls on a zeroed scratch tile keep the PE
            # busy while the first input DMAs land, so the HAM clock-gate
            # opens (1.2 -> 2.4 GHz) before the real matmuls start.
            scr = cpool.tile([128, 512], f16, tag="scr")
            nc.gpsimd.memset(scr[:], 0.0)
            wps = psE.tile([80, 1024], f32, tag="pse")
            for _ in range(N_WARM):
                nc.tensor.matmul(wps[:, 0:512], scr[:, 0:80], scr[:],
                                 start=True, stop=True)

            wts_t = cpool.tile([128, 400], f16, tag="wts")
            nc.sync.dma_start(wts_t[:], wts[:])
            wE_t = wts_t[:, 0:160]

            wav_t = wpool.tile([36, 2 * WAV_NCOL], f16, tag="wav")

            echunks = []
            for j in range(N_ECHUNK):
                et = epool.tile([128, ECHUNK + 1], f16, tag="eegchunk")
                lo = j * ECHUNK
                nc.sync.dma_start(et[:], eegP[:, lo:lo + ECHUNK + 1])
                echunks.append(et)
                if j == 1:
                    nc.sync.dma_start(wav_t[:], wavP[:])

            mE = mpool.tile([80, 2], f32, tag="mE")       # direct fp32 pairs
            mE16 = mpool.tile([80, 16], f16, tag="mE16")  # staged fp16 pairs
            mA = mpool.tile([120, 22], f16, tag="mA")
            mB = mpool.tile([120, 22], f16, tag="mB")

            Copy = mybir.ActivationFunctionType.Copy
            X = mybir.AxisListType.X

            def staged_reduce(ps, nvalid, mt, col0, P):
                """ACT casts PSUM->fp16 SBUF; DVE reduces with a wide stage-1
                output (8 cols per 1024) to hit the 2-byte 2x mode."""
                st = spool.tile([120, 1024], f16, tag="stg")
                nc.scalar.activation(st[:P, :nvalid], ps[:, :nvalid], Copy)
                nfull = (nvalid // 128) * 128
                cols = nvalid // 128
                nc.vector.reduce_max(
                    mt[:, col0:col0 + cols],
                    st[:P, :nfull].rearrange("p (c n) -> p c n", n=128), axis=X)
                if nvalid > nfull:
                    nc.vector.reduce_max(mt[:, col0 + cols:col0 + cols + 1],
                                         st[:P, nfull:nvalid], axis=X)
                    cols += 1
                return cols

            # eeg: 4 psum pairs; alternate direct-fp32 DVE reduce with the
            # ACT-staged fp16 path to balance the two engines
            for p in range(4):
                ch = echunks[p]
                ps = psE.tile([80, 1024], f32, tag="pse")
                for g in range(2):
                    for j in range(2):
                        lo = j * 512
                        nc.tensor.matmul(ps[:, lo:lo + 512],
                                         wE_t[:, 80 * g:80 * g + 80],
                                         ch[:, lo + g:lo + g + 512],
                                         start=(g == 0), stop=(g == 1))
                if p % 2 == 0:
                    nc.vector.reduce_max(mE[:, p // 2:p // 2 + 1], ps[:], axis=X)
                else:
                    staged_reduce(ps, 1024, mE16, 8 * (p // 2), 80)

            # wav streams: all pairs via the ACT-staged fp16 path
            for si, mt in enumerate([mA, mB]):
                for p in range(3):
                    ps = psW.tile([120, 1024], f32, tag="psw")
                    nvalid = 1024 if p < 2 else 683
                    for j in range(2):
                        n0 = si * WAV_NCOL + (2 * p + j) * 512
                        nn = min(512, (si + 1) * WAV_NCOL - n0)
                        nc.tensor.matmul(ps[:, j * 512:j * 512 + nn],
                                         wts_t[0:36, 160 + 120 * si:280 + 120 * si],
                                         wav_t[:, n0:n0 + nn],
                                         start=True, stop=True)
                    staged_reduce(ps, nvalid, mt, 8 * p, 120)

            fin = mpool.tile([128, 3], f32, tag="fin")
            finh = mpool.tile([128, 3], f16, tag="finh")
            tE = mpool.tile([80, 2], f32, tag="tE")
            nc.gpsimd.memset(fin[:], 0.0)
            nc.vector.reduce_max(tE[:, 0:1], mE[:], axis=X)
            nc.vector.reduce_max(finh[0:80, 0:1], mE16[:], axis=X)
            nc.vector.tensor_copy(tE[:, 1:2], finh[0:80, 0:1])
            nc.vector.reduce_max(fin[0:80, 0:1], tE[:], axis=X)
            nc.vector.reduce_max(finh[0:120, 1:2], mA[:], axis=X)
            nc.vector.reduce_max(finh[0:120, 2:3], mB[:], axis=X)
            nc.vector.tensor_copy(fin[0:120, 1:3], finh[0:120, 1:3])

            nc.sync.dma_start(out[:], fin[:])

    nc.compile()
    return nc


def _get_nc():
    if "nc" not in _NC_CACHE:
        _NC_CACHE["nc"] = _build_nc()
    return _NC_CACHE["nc"]


# --------------------------------------------------------------------------
# entry point
# --------------------------------------------------------------------------

def _prepare_in_maps(x, mu, projA_w, projB_w, conv_w):
    x = np.asarray(x, np.float32)
    eeg = np.ascontiguousarray(x[0, 0, 1:17, :]).astype(np.float16)
    zt = np.zeros(64, np.float32)
    w_padA = np.concatenate([np.zeros(7, np.float32), x[0, 0, 0, :], zt]
                            ).astype(np.float16)
    w_padB = np.concatenate([np.zeros(7, np.float32), x[0, 0, 17, :], zt]
                            ).astype(np.float16)

    conv_w = np.asarray(conv_w)
    E_A = _composite_wav_weights(mu, projA_w, conv_w[0])
    E_B = _composite_wav_weights(mu, projB_w, conv_w[2])
    wts_np = np.zeros((128, 400), np.float16)
    wts_np[:, 0:160] = _eeg_lhsT(conv_w[1])
    wts_np[0:36, 160:280] = _wav_lhsT(E_A)
    wts_np[0:36, 280:400] = _wav_lhsT(E_B)

    in_maps = []
    for k in range(NCORES):
        wavp = np.concatenate([_wav_phases(w_padA, k), _wav_phases(w_padB, k)],
                              axis=1)
        in_maps.append({
            "eegP": np.ascontiguousarray(_eeg_phases(eeg, k)),
            "wavP": np.ascontiguousarray(wavp),
            "wts": wts_np,
        })
    return in_maps


def _head(percore, conv_b, fc1_w, fc1_b, fc2_w, fc2_b):
    m = percore.max(axis=0).astype(np.float64)
    eeg_o = m[0:80].reshape(10, 8).max(axis=1)
    wavA_o = m[80:200].reshape(10, 12).max(axis=1)
    wavB_o = m[200:320].reshape(10, 12).max(axis=1)
    conv_b = np.asarray(conv_b, np.float64)
    f = np.concatenate([np.maximum(wavA_o + conv_b[0], 0.0),
                        np.maximum(eeg_o + conv_b[1], 0.0),
                        np.maximum(wavB_o + conv_b[2], 0.0)])
    h = 1.0 / (1.0 + np.exp(-(f @ np.asarray(fc1_w, np.float64).T
                              + np.asarray(fc1_b, np.float64))))
    o = 1.0 / (1.0 + np.exp(-(h @ np.asarray(fc2_w, np.float64).T
                              + np.asarray(fc2_b, np.float64))))
    return o[None, :].astype(np.float32)


def _percore_from_out(arr):
    """Device 'out' [128,3] fp32 -> flat [320] (eeg 80, wavA 120, wavB 120)."""
    arr = np.asarray(arr)
    return np.concatenate([arr[0:80, 0], arr[0:120, 1], arr[0:120, 2]])


def kernel(x, mu, projA_w, projB_w, conv_w, conv_b, fc1_w, fc1_b, fc2_w, fc2_b):
    global LAST_RESULT
    in_maps = _prepare_in_maps(x, mu, projA_w, projB_w, conv_w)
    nc = _get_nc()

    if os.environ.get("KERNEL_USE_SIM"):
        # sim mode for correctness checking without hardware
        from concourse.bass_interp import CoreSim
        percore = np.zeros((NCORES, 320), np.float32)
        for k in range(NCORES):
            sim = CoreSim(nc)
            for name, arr in in_maps[k].items():
                sim.tensor(name)[:] = arr
            sim.simulate()
            percore[k] = _percore_from_out(sim.tensor("out"))
    else:
        from concourse.bass_utils import run_bass_kernel_spmd
        trace = bool(os.environ.get("KERNEL_TRACE"))
        res = run_bass_kernel_spmd(nc, in_maps, list(range(NCORES)),
                                   trace=trace)
        LAST_RESULT = res
        percore = np.stack([_percore_from_out(res.results[k]["out"])
                            for k in range(NCORES)])

    return _head(percore, conv_b, fc1_w, fc1_b, fc2_w, fc2_b)
